# revision 1
# baseline (speedup 1.0000x reference)
"""Causal self-attention (GQA, qk-RMS-norm, RoPE) Trainium2 Bass kernel.

Sharding (8 cores): batch (2) x kv-head-group (4).  Core c handles batch
b = c // 4 and kv head g = c % 4 (with its 4 query heads 4g..4g+3).
Each core computes a (T, D) partial of the output projection (Wproj input
dim is split across the 4 tensor-parallel cores); the host sums the 4
partials per batch element.

Device-side formulation (per core):
  - host passes x^T, [Wq|Wk|Wv]^T slice, Wproj^T slice, rope tables (all
    bf16 except rope tables/consts)
  - QKV proj:  psum[t_tile, 384] = sum_k xT_tile^T . wqkvT_tile  (bf16)
  - RoPE applied to raw q/k straight out of PSUM (rotation commutes with
    the rms-norm scaling); the q rms-norm scale is applied in SBUF (cast
    to bf16 on the way), the k rms-norm scale and the 1/sqrt(hd) score
    scale are folded into the per-partition `scale` operand of the Exp.
  - scores are computed transposed (S^T[tk, tq]) so no softmax-max pass is
    needed (|score| <= 8 after rms norm) and A.V needs no transposes:
        S^T = kT^T . qT      (kT replicated into both 64-partition halves;
                              head pairs run concurrently via PE row groups)
        U = exp(0.125 * rstd_k[tk] * S^T)  (ScalarE, straight from PSUM,
                                            cast to bf16)
        Y^T[pair] += ([V|0] / [0|V])^T . U  (heads packed 2-per-psum-bank
                                             via zero-padded V operands)
        den[pair] += block-ones^T . U       (denominators land broadcast
                                             across each head's 64 rows)
  - causal masking: tk-tiles strictly above the diagonal are skipped
    entirely (column slicing), the single diagonal 128x128 block of u is
    zeroed by a multiplicative {0,1} mask on DVE after the exp (so the
    PE -> ScalarE exp chain never waits on masking).
  - y normalized by the reciprocal of the broadcast denominators, written
    as yT (d' on partitions) which directly feeds the projection matmuls.
  - emission order: all qkv groups (pipelined through one 4-bank psum
    tile), then attention chunks with the projection of chunk c emitted
    right after chunk c, sharing the denominator psum slots so it overlaps
    chunk c+1's score/exp work.  Cost-model timeline: ~198 us/core.
"""

import os
import sys

import ml_dtypes
import numpy as np

for _p in ("/opt/trn_rl_repo", "/root/.axon_site/_ro/trn_rl_repo"):
    if _p not in sys.path and os.path.isdir(_p):
        sys.path.append(_p)

import concourse.bass as bass
import concourse.bacc as bacc_mod
import concourse.mybir as mybir
import concourse.tile as tile
from concourse.bass import ts
from concourse.bass_utils import run_bass_kernel_spmd

F32 = mybir.dt.float32
BF16 = mybir.dt.bfloat16

B, T, D = 2, 2048, 1024
H, HKV, HD = 16, 4, 64
NH = H // HKV            # 4 local q heads per core
P = 128
NT = T // P              # 16 t tiles
ND = D // P              # 8 d tiles
GT = 4                   # t-tiles per qkv group
NG = NT // GT            # 4 groups
CW = 512                 # tq chunk width
NCHUNK = T // CW         # 4
QKV_W = NH * HD + 2 * HD  # 384 = q(256) | k(64) | v(64)
EPS = float(np.finfo(np.float32).eps)
SCALE = float(HD) ** -0.5
ROPE_BASE = 10000.0

# consts layout (columns of the f32 "consts" input, [P, NCONST])
C_TRI = 0      # 0:256   additive causal mask for diagonal blocks, 2 copies
C_QG = 256     # 256:272 q_gain per head, replicated GT times ([P, GT, 4])
C_EPS = 272    # 272:273 eps
NCONST = 273

# blk layout (columns of the bf16 "blk" input, [P, 512])
#   0:256   [0 x64 | 1 x64 | 0 x128]: slices give [1s|0s] / [0s|1s] lhsT
#           for the broadcast denominator matmuls
#   256:384 identity (bf16) for PE transposes
#   384:640 multiplicative causal tri mask {0,1}, 2 copies
BLK_W = 640


def _build_bass(debug=False):
    nc = bacc_mod.Bacc(trn_type="TRN2")

    xT_d = nc.dram_tensor("xT", [D, T], BF16, kind="ExternalInput")
    wqkv_d = nc.dram_tensor("wqkvT", [D, QKV_W], BF16, kind="ExternalInput")
    wproj_d = nc.dram_tensor("wprojT", [NH * HD, D], BF16, kind="ExternalInput")
    cos_d = nc.dram_tensor("cosT", [T, NH + 1, HD // 2], F32, kind="ExternalInput")
    sin_d = nc.dram_tensor("sinT", [T, NH + 1, HD // 2], F32, kind="ExternalInput")
    consts_d = nc.dram_tensor("consts", [P, NCONST], F32, kind="ExternalInput")
    blk_d = nc.dram_tensor("blk", [P, BLK_W], BF16, kind="ExternalInput")
    out_d = nc.dram_tensor("outp", [T, D], F32, kind="ExternalOutput")
    if debug:
        dbg_qT = nc.dram_tensor("dbg_qT", [P, 2, T], BF16, kind="ExternalOutput")
        dbg_kT = nc.dram_tensor("dbg_kT", [P, T], BF16, kind="ExternalOutput")
        dbg_v = nc.dram_tensor("dbg_v", [P, NT, 192], BF16, kind="ExternalOutput")
        dbg_rk = nc.dram_tensor("dbg_rk", [P, NT], F32, kind="ExternalOutput")
        dbg_yT = nc.dram_tensor("dbg_yT", [P, 2, T], BF16, kind="ExternalOutput")
        dbg_u = nc.dram_tensor("dbg_u", [P, NH, CW], BF16, kind="ExternalOutput")
        dbg_dr = nc.dram_tensor(
            "dbg_dr", [NCHUNK, 2, P, CW], F32, kind="ExternalOutput"
        )
        dbg_y = nc.dram_tensor(
            "dbg_y", [NCHUNK, 2, P, CW], F32, kind="ExternalOutput"
        )

    with tile.TileContext(nc) as tc:
        _b = lambda k, d: int(os.environ.get(f"KB_{k}", str(d)))
        with (
            tc.tile_pool(name="singles", bufs=1) as singles,
            tc.tile_pool(name="xg", bufs=4) as xg_pool,
            tc.tile_pool(name="qk", bufs=_b("QK", 2)) as qk_pool,
            tc.tile_pool(name="stat", bufs=_b("ST", 2)) as stat_pool,
            tc.tile_pool(name="u", bufs=_b("U", 6)) as u_pool,
            tc.tile_pool(name="r", bufs=2) as r_pool,
            tc.tile_pool(name="ob", bufs=_b("OB", 2)) as ob_pool,
        ):
            # ---------------- persistent SBUF ----------------
            wqkv_sb = singles.tile([P, ND, QKV_W], BF16)
            wproj_sb = singles.tile([P, 2, D], BF16)
            cos_sb = singles.tile([P, NT, NH + 1, HD // 2], F32)
            sin_sb = singles.tile([P, NT, NH + 1, HD // 2], F32)
            consts_sb = singles.tile([P, NCONST], F32)
            blk_sb = singles.tile([P, BLK_W], BF16)
            # pair pr: head 2pr at partitions 0:64, head 2pr+1 at 64:128
            qT_sb = singles.tile([P, 2, T], BF16)
            kT_sb = singles.tile([P, T], BF16)      # replicated into both halves
            v_sb = singles.tile([P, NT, 192], BF16)  # cols 64:128 hold V
            yT_sb = singles.tile([P, 2, T], BF16)
            rstdk_sb = singles.tile([P, NT], F32)   # 0.125 * rstd_k per tk tile

            nc.sync.dma_start(
                out=wqkv_sb, in_=wqkv_d[:].rearrange("(po pi) f -> pi po f", pi=P)
            )
            nc.sync.dma_start(
                out=wproj_sb, in_=wproj_d[:].rearrange("(po pi) f -> pi po f", pi=P)
            )
            nc.sync.dma_start(
                out=cos_sb, in_=cos_d[:].rearrange("(nt p) h f -> p nt h f", p=P)
            )
            nc.sync.dma_start(
                out=sin_sb, in_=sin_d[:].rearrange("(nt p) h f -> p nt h f", p=P)
            )
            nc.sync.dma_start(out=consts_sb, in_=consts_d[:])
            nc.sync.dma_start(out=blk_sb, in_=blk_d[:])
            nc.gpsimd.memset(v_sb, 0.0)

            qg4 = consts_sb[:, C_QG : C_QG + 16].rearrange("p (g h) -> p g h", g=GT)
            eps_ap = consts_sb[:, C_EPS : C_EPS + 1]
            ident = blk_sb[:, 256:384]
            tri2b = blk_sb[:, 384:640].rearrange("p (j f) -> p j f", j=2)

            def den_lhsT(h):
                # [1s|0s] for even heads (denominator broadcast over psum
                # rows 0:64), [0s|1s] for odd heads (rows 64:128)
                return blk_sb[:, 64:192] if h % 2 == 0 else blk_sb[:, 0:128]

            def emit_group(g, qkv_ps_pool, tr_ps_pool, qkv_tag="qkv"):
                """QKV projection + rms stats + rope + transposes for the
                4 t-tiles of group g.  The qkv psum is split into two
                2-bank sub-tiles so it can share slots with the score
                tiles in mixed emission order."""
                xg_sb = xg_pool.tile([P, ND, GT * P], BF16, tag="xg",
                                     name=f"xg{g}")
                for ik in range(ND):
                    nc.sync.dma_start(
                        out=xg_sb[:, ik, :],
                        in_=xT_d[ts(ik, P), ts(g, GT * P)],
                    )
                nqk = GT * (NH + 1)  # 20
                qk_raw = qk_pool.tile([P, nqk, HD], F32, tag="qkraw")
                if qkv_tag == "s":
                    # attention-interleaved mode: two 2-bank sub-tiles that
                    # fit the score-tile slots
                    for half in range(2):
                        qkv_ps = qkv_ps_pool.tile(
                            [P, 2, 512], F32, tag="s", name=f"qkv{g}_{half}"
                        )
                        for jj in range(2):
                            j = 2 * half + jj
                            for ik in range(ND):
                                nc.tensor.matmul(
                                    qkv_ps[:, jj, 0:QKV_W],
                                    lhsT=xg_sb[:, ik, ts(j, P)],
                                    rhs=wqkv_sb[:, ik, :],
                                    start=(ik == 0),
                                    stop=(ik == ND - 1),
                                )
                        nc.vector.tensor_copy(
                            qk_raw[
                                :, 2 * half * (NH + 1) : (2 * half + 2) * (NH + 1), :
                            ].rearrange("p n x -> p (n x)"),
                            qkv_ps[:, :, 0 : (NH + 1) * HD],
                        )
                        nc.vector.tensor_copy(
                            v_sb[:, ts(2 * g + half, 2), 64:128],
                            qkv_ps[:, :, 320:384],
                        )
                elif _b("QKV", 1) > 1:
                    # deep-buffered 2-bank sub-tiles: matmuls of later
                    # sub-tiles run while DVE drains earlier ones
                    for half in range(2):
                        qkv_ps = qkv_ps_pool.tile(
                            [P, 2, 512], F32, tag="qkv", name=f"qkv{g}_{half}"
                        )
                        for jj in range(2):
                            j = 2 * half + jj
                            for ik in range(ND):
                                nc.tensor.matmul(
                                    qkv_ps[:, jj, 0:QKV_W],
                                    lhsT=xg_sb[:, ik, ts(j, P)],
                                    rhs=wqkv_sb[:, ik, :],
                                    start=(ik == 0),
                                    stop=(ik == ND - 1),
                                )
                        nc.vector.tensor_copy(
                            qk_raw[
                                :, 2 * half * (NH + 1) : (2 * half + 2) * (NH + 1), :
                            ].rearrange("p n x -> p (n x)"),
                            qkv_ps[:, :, 0 : (NH + 1) * HD],
                        )
                        nc.vector.tensor_copy(
                            v_sb[:, ts(2 * g + half, 2), 64:128],
                            qkv_ps[:, :, 320:384],
                        )
                else:
                    qkv_ps = qkv_ps_pool.tile(
                        [P, GT, 512], F32, tag=qkv_tag, name=f"qkv{g}"
                    )
                    for j in range(GT):
                        for ik in range(ND):
                            nc.tensor.matmul(
                                qkv_ps[:, j, 0:QKV_W],
                                lhsT=xg_sb[:, ik, ts(j, P)],
                                rhs=wqkv_sb[:, ik, :],
                                start=(ik == 0),
                                stop=(ik == ND - 1),
                            )
                    # copy raw q|k out of psum into a tightly-packed tile:
                    # the (tile, head) dims collapse to one 20-wide dim,
                    # keeping every rope/stat AP in walrus' 3-dim TT limit
                    nc.vector.tensor_copy(
                        qk_raw.rearrange("p n x -> p (n x)"),
                        qkv_ps[:, :, 0 : (NH + 1) * HD],
                    )
                    # copy V out right away: it is the last psum reader, so
                    # the next group's matmuls can recycle the bank sooner
                    nc.vector.tensor_copy(
                        v_sb[:, ts(g, GT), 64:128], qkv_ps[:, :, 320:384]
                    )

                # rms statistics on raw q, k
                sq = qk_pool.tile([P, nqk, HD], F32, tag="sq")
                qk_flat = qk_raw.rearrange("p n x -> p (n x)")
                nc.vector.tensor_mul(
                    sq.rearrange("p n x -> p (n x)"), qk_flat, qk_flat
                )
                ssq = stat_pool.tile([P, nqk], F32, tag="ssq")
                nc.vector.reduce_sum(ssq, sq, axis=mybir.AxisListType.X)
                nc.scalar.activation(
                    out=ssq,
                    in_=ssq,
                    func=mybir.ActivationFunctionType.Sqrt,
                    bias=eps_ap,
                    scale=1.0 / HD,
                )
                rstd = stat_pool.tile([P, nqk], F32, tag="rstd")
                nc.vector.reciprocal(rstd, ssq)
                rstd4 = rstd.rearrange("p (g h) -> p g h", g=GT)
                # fold q_gain into the q rstds
                nc.vector.tensor_mul(rstd4[:, :, 0:NH], rstd4[:, :, 0:NH], qg4)
                # stash k rstd * SCALE for the exp
                nc.scalar.mul(
                    out=rstdk_sb[:, ts(g, GT)],
                    in_=rstd4[:, :, NH : NH + 1].rearrange("p g o -> p (g o)"),
                    mul=SCALE,
                )

                # rope in place on raw q|k (rotation commutes with rms scale)
                q1 = qk_raw[:, :, 0 : HD // 2]
                q2 = qk_raw[:, :, HD // 2 : HD]
                cg = cos_sb[:, ts(g, GT), :, :].rearrange("p g h x -> p (g h) x")
                sg = sin_sb[:, ts(g, GT), :, :].rearrange("p g h x -> p (g h) x")
                t_a = qk_pool.tile([P, nqk, HD // 2], F32, tag="ta")
                t_b = qk_pool.tile([P, nqk, HD // 2], F32, tag="tb")
                t_c = qk_pool.tile([P, nqk, HD // 2], F32, tag="tc")
                t_d = qk_pool.tile([P, nqk, HD // 2], F32, tag="td")
                nc.vector.tensor_mul(t_a, q1, cg)
                nc.vector.tensor_mul(t_b, q2, sg)
                nc.vector.tensor_mul(t_c, q1, sg)
                nc.vector.tensor_mul(t_d, q2, cg)
                nc.vector.tensor_add(q1, t_a, t_b)
                nc.vector.tensor_sub(q2, t_d, t_c)

                # scale q heads by rstd (casting to bf16); copy k unscaled
                # (its rms scale is folded into the exp)
                qk_c = qk_pool.tile([P, nqk, HD], BF16, tag="qkc")
                for j in range(GT):
                    for h in range(NH):
                        i = j * (NH + 1) + h
                        nc.vector.tensor_scalar_mul(
                            out=qk_c[:, i, :],
                            in0=qk_raw[:, i, :],
                            scalar1=rstd[:, i : i + 1],
                        )
                    ik_ = j * (NH + 1) + NH
                    nc.vector.tensor_copy(qk_c[:, ik_, :], qk_raw[:, ik_, :])

                # bf16 transposes: q head-pairs and k
                for j in range(GT):
                    it = g * GT + j
                    i0 = j * (NH + 1)
                    for pr in range(2):
                        trq = tr_ps_pool.tile([P, P], BF16, tag="trq" if tr_ps_pool is not qkv_ps_pool else "s")
                        nc.tensor.transpose(
                            trq, qk_c[:, i0 + 2 * pr : i0 + 2 * pr + 2, :], ident
                        )
                        nc.vector.tensor_copy(qT_sb[:, pr, ts(it, P)], trq)
                    trk = tr_ps_pool.tile([P, P], BF16, tag="trq" if tr_ps_pool is not qkv_ps_pool else "s")
                    nc.tensor.transpose(trk[0:64, :], qk_c[:, i0 + NH, :], ident)
                    nc.vector.tensor_copy(kT_sb[0:64, ts(it, P)], trk[0:64, :])
                    # replicate kT into the upper partition half for the
                    # odd-head row-group score matmuls
                    nc.sync.dma_start(
                        out=kT_sb[64:128, ts(it, P)], in_=kT_sb[0:64, ts(it, P)]
                    )

            den_holder = [None]

            def emit_chunk(c, s_ps_pool, y_ps_pool, den_ps_pool):
                den_holder[0] = den_ps_pool
                """Attention for tq chunk c (needs groups 0..c done)."""
                ntk = (c + 1) * (CW // P)
                y_ps = [
                    y_ps_pool.tile([P, CW], F32, tag="y", name=f"y_c{c}_{pr}")
                    for pr in range(2)
                ]
                den_ps = [
                    den_ps_pool.tile([P, CW], F32, tag="den", name=f"den_c{c}_{pr}")
                    for pr in range(2)
                ]
                for tk in range(ntk):
                    dj = tk - 4 * c  # >= 0 on the diagonal block
                    lo = P * dj if dj >= 0 else 0
                    u = u_pool.tile([P, NH, CW], BF16, tag="u")
                    for pr in range(2):
                        s_ps = s_ps_pool.tile([P, 2, CW], F32, tag="s")
                        for hh in range(2):
                            nc.tensor.matmul(
                                s_ps[:, hh, lo:],
                                lhsT=kT_sb[64 * hh : 64 * (hh + 1), ts(tk, P)],
                                rhs=qT_sb[
                                    64 * hh : 64 * (hh + 1),
                                    pr,
                                    c * CW + lo : (c + 1) * CW,
                                ],
                                start=True,
                                stop=True,
                            )
                        nc.scalar.activation(
                            out=u[:, 2 * pr : 2 * pr + 2, lo:],
                            in_=s_ps[:, :, lo:],
                            func=mybir.ActivationFunctionType.Exp,
                            scale=rstdk_sb[:, tk : tk + 1],
                        )
                        if dj >= 0:
                            # multiplicative causal mask on the diagonal
                            # 128-block of u; runs on DVE so the PE->ACT
                            # exp chain is never blocked on it
                            nc.vector.tensor_mul(
                                u[:, 2 * pr : 2 * pr + 2, lo : lo + P],
                                u[:, 2 * pr : 2 * pr + 2, lo : lo + P],
                                tri2b,
                            )
                    if debug and c == 0 and tk == 0:
                        nc.sync.dma_start(out=dbg_u[:], in_=u)
                    horder = (0, 2, 1, 3) if os.environ.get("KB_HORD", "0") == "1" else (0, 1, 2, 3)
                    for h in horder:
                        pr, hh = divmod(h, 2)
                        # even head -> [V|0], odd -> [0|V]
                        vop = v_sb[:, tk, 64:192] if hh == 0 else v_sb[:, tk, 0:128]
                        nc.tensor.matmul(
                            y_ps[pr][:, lo:],
                            lhsT=vop,
                            rhs=u[:, h, lo:],
                            start=(tk == 0 and hh == 0),
                            stop=(tk == ntk - 1 and hh == 1),
                            skip_group_check=True,
                        )
                        nc.tensor.matmul(
                            den_ps[pr][:, lo:],
                            lhsT=den_lhsT(h),
                            rhs=u[:, h, lo:],
                            start=(tk == 0 and hh == 0),
                            stop=(tk == ntk - 1 and hh == 1),
                            skip_group_check=True,
                        )

                for pr in range(2):
                    dr = r_pool.tile([P, CW], F32, tag="dr")
                    nc.vector.reciprocal(dr, den_ps[pr])
                    if debug:
                        nc.sync.dma_start(out=dbg_dr[c, pr], in_=dr)
                        ystg = r_pool.tile([P, CW], F32, tag="ystg")
                        nc.vector.tensor_copy(ystg, y_ps[pr])
                        nc.sync.dma_start(out=dbg_y[c, pr], in_=ystg)
                    nc.vector.tensor_mul(
                        yT_sb[:, pr, ts(c, CW)], y_ps[pr], dr
                    )

            def emit_proj(c, proj_ps_pool):
                """Output projection for t-tiles 4c..4c+3.  proj_ps_pool may
                be the y pool (tag-shared slots) so this can interleave with
                the next chunk's attention."""
                for j in range(GT):
                    it = c * GT + j
                    ob = ob_pool.tile([P, D], F32, tag="ob")
                    for nh_ in range(2):
                        pj = proj_ps_pool.tile(
                            [P, CW], F32,
                            tag="den" if proj_ps_pool is den_holder[0] else "y",
                            name=f"pj{c}_{j}_{nh_}",
                        )
                        for kt in range(2):
                            nc.tensor.matmul(
                                pj,
                                lhsT=yT_sb[:, kt, ts(it, P)],
                                rhs=wproj_sb[:, kt, ts(nh_, CW)],
                                start=(kt == 0),
                                stop=(kt == 1),
                            )
                        if nh_ == 0:
                            nc.scalar.copy(ob[:, 0:CW], pj)
                        else:
                            nc.vector.tensor_copy(ob[:, CW:D], pj)
                    nc.sync.dma_start(out=out_d[ts(it, P), 0:CW], in_=ob[:, 0:CW])
                    nc.sync.dma_start(out=out_d[ts(it, P), CW:D], in_=ob[:, CW:D])

            order = os.environ.get("KERNEL_ORDER", "phases")
            phases = int(os.environ.get("KERNEL_PHASES", "3"))
            if order == "segments":
                # interleaved emission: group g -> chunk g -> proj g, with
                # per-segment PSUM pools
                for seg in range(NG):
                    with (
                        tc.tile_pool(
                            name=f"qkv_ps{seg}", bufs=1, space="PSUM"
                        ) as qkv_ps_pool,
                        tc.tile_pool(
                            name=f"tr_ps{seg}", bufs=2, space="PSUM"
                        ) as tr_ps_pool,
                    ):
                        emit_group(seg, qkv_ps_pool, tr_ps_pool)
                    with (
                        tc.tile_pool(
                            name=f"s_ps{seg}", bufs=2, space="PSUM"
                        ) as s_ps_pool,
                        tc.tile_pool(
                            name=f"y_ps{seg}", bufs=2, space="PSUM"
                        ) as y_ps_pool,
                        tc.tile_pool(
                            name=f"den_ps{seg}", bufs=2, space="PSUM"
                        ) as den_ps_pool,
                    ):
                        emit_chunk(seg, s_ps_pool, y_ps_pool, den_ps_pool)
                    with tc.tile_pool(
                        name=f"proj_ps{seg}", bufs=2, space="PSUM"
                    ) as proj_ps_pool:
                        emit_proj(seg, proj_ps_pool)
            elif order == "mix":
                # fully interleaved: groups and chunks share the score-tile
                # psum slots; proj shares the den slots.  One set of pools
                # spans the whole kernel so cross-phase overlap is limited
                # only by real data deps and slot contention.
                with (
                    tc.tile_pool(name="s_ps", bufs=2, space="PSUM") as s_ps_pool,
                    tc.tile_pool(name="y_ps", bufs=2, space="PSUM") as y_ps_pool,
                    tc.tile_pool(name="den_ps", bufs=2, space="PSUM") as den_ps_pool,
                ):
                    emit_group(0, s_ps_pool, s_ps_pool, qkv_tag="s")
                    emit_group(1, s_ps_pool, s_ps_pool, qkv_tag="s")
                    for c in range(NCHUNK):
                        emit_chunk(c, s_ps_pool, y_ps_pool, den_ps_pool)
                        if c + 2 < NG:
                            emit_group(c + 2, s_ps_pool, s_ps_pool, qkv_tag="s")
                        emit_proj(c, den_ps_pool)
            else:
                # groups 0-1 up front in their own pools; groups 2-3 are
                # woven into the attention stream on the score-tile slots
                # (chunk c only needs groups <= c, so g2 goes after chunk 0
                # and g3 after chunk 1); proj(c) rides the den slots
                nfront = int(os.environ.get("KERNEL_NFRONT", "4"))
                with (
                    tc.tile_pool(name="qkv_ps", bufs=_b("QKV", 1), space="PSUM") as qkv_ps_pool,
                    tc.tile_pool(name="tr_ps", bufs=_b("TR", 4), space="PSUM") as tr_ps_pool,
                ):
                    for g in range(nfront):
                        emit_group(g, qkv_ps_pool, tr_ps_pool)
                if phases >= 2:
                    with (
                        tc.tile_pool(name="s_ps", bufs=2, space="PSUM") as s_ps_pool,
                        tc.tile_pool(name="y_ps", bufs=2, space="PSUM") as y_ps_pool,
                        tc.tile_pool(
                            name="den_ps", bufs=2, space="PSUM"
                        ) as den_ps_pool,
                    ):
                        for c in range(NCHUNK):
                            emit_chunk(c, s_ps_pool, y_ps_pool, den_ps_pool)
                            if c + nfront < NG:
                                emit_group(
                                    c + nfront, s_ps_pool, s_ps_pool, qkv_tag="s"
                                )
                            if phases >= 3:
                                emit_proj(c, den_ps_pool)

            if debug:
                nc.sync.dma_start(out=dbg_qT[:], in_=qT_sb)
                nc.sync.dma_start(out=dbg_kT[:], in_=kT_sb)
                nc.sync.dma_start(out=dbg_v[:], in_=v_sb)
                nc.sync.dma_start(out=dbg_rk[:], in_=rstdk_sb)
                nc.sync.dma_start(out=dbg_yT[:], in_=yT_sb)

    nc.finalize()
    return nc


_NC_CACHE = {}


def _get_nc(debug=False):
    key = "dbg" if debug else "nc"
    if key not in _NC_CACHE:
        _NC_CACHE[key] = _build_bass(debug=debug)
    return _NC_CACHE[key]


def _make_consts(q_gain_local):
    consts = np.zeros((P, NCONST), dtype=np.float32)
    pi = np.arange(P)
    # additive causal mask for the diagonal block: 0 where tq >= tk (f >= p)
    madd = np.where(np.arange(P)[None, :] >= pi[:, None], 0.0, -1e30).astype(
        np.float32
    )
    consts[:, C_TRI : C_TRI + 128] = madd
    consts[:, C_TRI + 128 : C_TRI + 256] = madd
    consts[:, C_QG : C_QG + 16] = np.tile(
        np.asarray(q_gain_local, np.float32)[None, :], (P, GT)
    )
    consts[:, C_EPS] = EPS
    return consts


def _make_blk():
    blk = np.zeros((P, BLK_W), dtype=ml_dtypes.bfloat16)
    blk[:, 64:128] = 1.0
    blk[:, 256:384] = np.eye(P, dtype=np.float32).astype(ml_dtypes.bfloat16)
    tri = (np.arange(P)[None, :] >= np.arange(P)[:, None]).astype(np.float32)
    blk[:, 384:512] = tri.astype(ml_dtypes.bfloat16)
    blk[:, 512:640] = tri.astype(ml_dtypes.bfloat16)
    return blk


def _rope_tables():
    inv = 1.0 / (
        ROPE_BASE ** (np.arange(0, HD, 2, dtype=np.float32) / HD)
    )
    f = np.arange(T, dtype=np.float32)[:, None] * inv[None, :]
    cos = np.cos(f).astype(np.float32)
    sin = np.sin(f).astype(np.float32)
    # replicate across the 4 q heads + 1 k head (walrus rejects zero-step
    # broadcast APs in TensorTensor, so the broadcast happens host-side)
    cos5 = np.ascontiguousarray(
        np.broadcast_to(cos[:, None, :], (T, NH + 1, HD // 2))
    )
    sin5 = np.ascontiguousarray(
        np.broadcast_to(sin[:, None, :], (T, NH + 1, HD // 2))
    )
    return cos5, sin5


def _make_in_maps(x, Wq, Wk, Wv, Wproj, q_gain):
    x = np.ascontiguousarray(np.asarray(x, np.float32))
    Wq = np.asarray(Wq, np.float32)
    Wk = np.asarray(Wk, np.float32)
    Wv = np.asarray(Wv, np.float32)
    Wproj = np.asarray(Wproj, np.float32)
    q_gain = np.asarray(q_gain, np.float32)
    cos, sin = _rope_tables()
    bf16 = ml_dtypes.bfloat16
    xTs = [np.ascontiguousarray(x[b].T.astype(bf16)) for b in range(B)]
    blk = _make_blk()
    kvw = HKV * HD  # 256 per-core q slice width
    in_maps = []
    for core in range(8):
        b, g = divmod(core, HKV)
        wq = Wq[g * kvw : (g + 1) * kvw]
        wk = Wk[g * HD : (g + 1) * HD]
        wv = Wv[g * HD : (g + 1) * HD]
        wqkvT = np.ascontiguousarray(np.concatenate([wq, wk, wv], 0).T.astype(bf16))
        wprojT = np.ascontiguousarray(
            Wproj[:, g * kvw : (g + 1) * kvw].T.astype(bf16)
        )
        consts = _make_consts(q_gain[g * NH : (g + 1) * NH])
        in_maps.append(
            {
                "xT": xTs[b],
                "wqkvT": wqkvT,
                "wprojT": wprojT,
                "cosT": cos,
                "sinT": sin,
                "consts": consts,
                "blk": blk,
            }
        )
    return in_maps


def run_sharded(inputs, trace=False, debug=False, **kwargs):
    """Run the SPMD kernel; returns (full_output, BassKernelResults)."""
    in_maps = _make_in_maps(**inputs)
    res = run_bass_kernel_spmd(
        _get_nc(debug=debug), in_maps, core_ids=list(range(8)), trace=trace,
        **kwargs
    )
    out = np.zeros((B, T, D), np.float32)
    for core in range(8):
        out[core // HKV] += res.results[core]["outp"]
    return out, res


def kernel(x, Wq, Wk, Wv, Wproj, q_gain):
    out, _ = run_sharded(
        dict(x=x, Wq=Wq, Wk=Wk, Wv=Wv, Wproj=Wproj, q_gain=q_gain)
    )
    return out



# revision 2
# speedup vs baseline: 1.1768x; 1.1768x over previous
"""Causal self-attention (GQA, qk-RMS-norm, RoPE) Trainium2 Bass kernel.

Sharding (8 cores): batch (2) x kv-head-group (4).  Core c handles batch
b = c // 4 and kv head g = c % 4 (with its 4 query heads 4g..4g+3).
Each core computes a (T, D) partial of the output projection (Wproj input
dim is split across the 4 tensor-parallel cores); the host sums the 4
partials per batch element.

Device-side formulation (per core):
  - host passes x^T, [Wq|Wk|Wv]^T slice, Wproj^T slice, rope tables (all
    bf16)
  - QKV proj:  psum[t_tile, 384] = sum_k xT_tile^T . wqkvT_tile  (bf16)
  - raw q|k copied out of PSUM as bf16; RoPE applied in bf16 (4x DVE
    modes); rms stats from the bf16 squares (f32 accumulate); the q
    rms-norm scale applied in place, the k rms scale and 1/sqrt(hd)
    folded into the per-partition `scale` operand of the Exp.
  - scores computed transposed (S^T[tk, tq]) so no softmax-max pass is
    needed (|score| <= 8 after rms norm) and A.V needs no transposes:
        S^T = kT^T . qT      (kT replicated into both 64-partition halves;
                              head pairs share one 2-bank psum tile)
        U = exp(0.125 * rstd_k[tk] * S^T)  (ScalarE, psum -> bf16 sbuf)
        [Y^T; den] += [V | 1]^T . U        (denominator folded into the
                                            A.V matmul: psum rows 0:64
                                            hold y^T, rows 64:128 the
                                            broadcast denominator -- one
                                            matmul per head per k-tile)
  - causal masking: tk-tiles strictly above the diagonal are skipped
    entirely (column slicing), the single diagonal 128x128 block of u is
    zeroed by a multiplicative {0,1} mask on DVE after the exp.
  - normalize: dr = 1/den (DVE, psum cross-partition read), then
    yT = y * dr written bf16; both use the PSUM-operand exemption from
    the equal-base-partition TensorTensor rule.
  - output projection per 4-t-tile chunk rides the y psum slots; psum
    drained by DVE to a bf16 staging tile and DMA'd out (host sums the
    4 TP partials in f32).
"""

import os
import sys

import ml_dtypes
import numpy as np

for _p in ("/opt/trn_rl_repo", "/root/.axon_site/_ro/trn_rl_repo"):
    if _p not in sys.path and os.path.isdir(_p):
        sys.path.append(_p)

import concourse.bass as bass
import concourse.bacc as bacc_mod
import concourse.mybir as mybir
import concourse.tile as tile
from concourse.bass import ts
from concourse.bass_utils import run_bass_kernel_spmd

F32 = mybir.dt.float32
BF16 = mybir.dt.bfloat16

B, T, D = 2, 2048, 1024
H, HKV, HD = 16, 4, 64
NH = H // HKV            # 4 local q heads per core
P = 128
NT = T // P              # 16 t tiles
ND = D // P              # 8 d tiles
GT = 4                   # t-tiles per qkv group
NG = NT // GT            # 4 groups
CW = 512                 # tq chunk width
NCHUNK = T // CW         # 4
QKV_W = NH * HD + 2 * HD  # 384 = q(256) | k(64) | v(64)
EPS = float(np.finfo(np.float32).eps)
SCALE = float(HD) ** -0.5
ROPE_BASE = 10000.0

# consts layout (columns of the f32 "consts" input, [P, NCONST])
C_QG = 0       # 0:16 q_gain per head, replicated GT times ([P, GT, 4])
C_EPS = 16     # 16:17 eps
NCONST = 17

# blk layout (columns of the bf16 "blk" input, [P, BLK_W])
#   0:128   identity (bf16) for PE transposes
#   128:384 multiplicative causal tri mask {0,1}, 2 copies
BLK_W = 384


def _build_bass(debug=False):
    nc = bacc_mod.Bacc(trn_type="TRN2")

    xT_d = nc.dram_tensor("xT", [D, T], BF16, kind="ExternalInput")
    wqkv_d = nc.dram_tensor("wqkvT", [D, QKV_W], BF16, kind="ExternalInput")
    wproj_d = nc.dram_tensor("wprojT", [NH * HD, D], BF16, kind="ExternalInput")
    cos_d = nc.dram_tensor("cosT", [T, NH + 1, HD // 2], BF16, kind="ExternalInput")
    sin_d = nc.dram_tensor("sinT", [T, NH + 1, HD // 2], BF16, kind="ExternalInput")
    consts_d = nc.dram_tensor("consts", [P, NCONST], F32, kind="ExternalInput")
    blk_d = nc.dram_tensor("blk", [P, BLK_W], BF16, kind="ExternalInput")
    out_d = nc.dram_tensor("outp", [T, D], BF16, kind="ExternalOutput")
    if debug:
        dbg_qT = nc.dram_tensor("dbg_qT", [P, 2, T], BF16, kind="ExternalOutput")
        dbg_kT = nc.dram_tensor("dbg_kT", [P, T], BF16, kind="ExternalOutput")
        dbg_v = nc.dram_tensor("dbg_v", [P, NT, 128], BF16, kind="ExternalOutput")
        dbg_rk = nc.dram_tensor("dbg_rk", [P, NT], F32, kind="ExternalOutput")
        dbg_yT = nc.dram_tensor("dbg_yT", [P, 2, T], BF16, kind="ExternalOutput")
        dbg_u = nc.dram_tensor("dbg_u", [P, NH, CW], BF16, kind="ExternalOutput")
        dbg_den = nc.dram_tensor(
            "dbg_den", [NCHUNK, NH, P, CW], F32, kind="ExternalOutput"
        )

    with tile.TileContext(nc) as tc:
        _b = lambda k, d: int(os.environ.get(f"KB_{k}", str(d)))
        with (
            tc.tile_pool(name="singles", bufs=1) as singles,
            tc.tile_pool(name="xg", bufs=4) as xg_pool,
            tc.tile_pool(name="qk", bufs=_b("QK", 2)) as qk_pool,
            tc.tile_pool(name="stat", bufs=_b("ST", 2)) as stat_pool,
            tc.tile_pool(name="u", bufs=_b("U", 6)) as u_pool,
            tc.tile_pool(name="r", bufs=2) as r_pool,
            tc.tile_pool(name="ob", bufs=_b("OB", 2)) as ob_pool,
        ):
            # ---------------- persistent SBUF ----------------
            wqkv_sb = singles.tile([P, ND, QKV_W], BF16)
            wproj_sb = singles.tile([P, 2, D], BF16)
            cos_sb = singles.tile([P, NT, NH + 1, HD // 2], BF16)
            sin_sb = singles.tile([P, NT, NH + 1, HD // 2], BF16)
            consts_sb = singles.tile([P, NCONST], F32)
            blk_sb = singles.tile([P, BLK_W], BF16)
            # pair pr: head 2pr at partitions 0:64, head 2pr+1 at 64:128
            qT_sb = singles.tile([P, 2, T], BF16)
            kT_sb = singles.tile([P, T], BF16)      # replicated into both halves
            v_sb = singles.tile([P, NT, 128], BF16)  # [V | ones]
            yT_sb = singles.tile([P, 2, T], BF16)
            rstdk_sb = singles.tile([P, NT], F32)   # 0.125 * rstd_k per tk tile

            nc.sync.dma_start(
                out=wqkv_sb, in_=wqkv_d[:].rearrange("(po pi) f -> pi po f", pi=P)
            )
            nc.sync.dma_start(out=consts_sb, in_=consts_d[:])
            nc.sync.dma_start(out=blk_sb, in_=blk_d[:])
            nc.sync.dma_start(
                out=cos_sb, in_=cos_d[:].rearrange("(nt p) h f -> p nt h f", p=P)
            )
            nc.sync.dma_start(
                out=sin_sb, in_=sin_d[:].rearrange("(nt p) h f -> p nt h f", p=P)
            )
            nc.sync.dma_start(
                out=wproj_sb, in_=wproj_d[:].rearrange("(po pi) f -> pi po f", pi=P)
            )
            nc.gpsimd.memset(v_sb[:, :, 64:128], 1.0)

            qg4 = consts_sb[:, C_QG : C_QG + 16].rearrange("p (g h) -> p g h", g=GT)
            eps_ap = consts_sb[:, C_EPS : C_EPS + 1]
            ident = blk_sb[:, 0:128]
            tri2b = blk_sb[:, 128:384].rearrange("p (j f) -> p j f", j=2)

            def emit_group(g, qkv_ps_pool, tr_ps_pool, qkv_tag="qkv"):
                """QKV projection + rms stats + rope + transposes for the
                4 t-tiles of group g."""
                woven = qkv_tag == "s"
                xg_sb = xg_pool.tile([P, ND, GT * P], BF16, tag="xg",
                                     name=f"xg{g}")
                for ik in range(ND):
                    nc.sync.dma_start(
                        out=xg_sb[:, ik, :],
                        in_=xT_d[ts(ik, P), ts(g, GT * P)],
                    )
                nqk = GT * (NH + 1)  # 20
                qk_raw = qk_pool.tile([P, nqk, HD], BF16, tag="qkraw")
                if woven or _b("QKV", 1) > 1:
                    # 2-bank sub-tiles (fit the score-tile slots when woven;
                    # deeper pipelining otherwise)
                    for half in range(2):
                        qkv_ps = qkv_ps_pool.tile(
                            [P, 2, 512], F32,
                            tag="s" if woven else "qkv",
                            name=f"qkv{g}_{half}",
                        )
                        for jj in range(2):
                            j = 2 * half + jj
                            for ik in range(ND):
                                nc.tensor.matmul(
                                    qkv_ps[:, jj, 0:QKV_W],
                                    lhsT=xg_sb[:, ik, ts(j, P)],
                                    rhs=wqkv_sb[:, ik, :],
                                    start=(ik == 0),
                                    stop=(ik == ND - 1),
                                )
                        nc.vector.tensor_copy(
                            qk_raw[
                                :, 2 * half * (NH + 1) : (2 * half + 2) * (NH + 1), :
                            ].rearrange("p n x -> p (n x)"),
                            qkv_ps[:, :, 0 : (NH + 1) * HD],
                        )
                        nc.vector.tensor_copy(
                            v_sb[:, ts(2 * g + half, 2), 0:64],
                            qkv_ps[:, :, 320:384],
                        )
                else:
                    qkv_ps = qkv_ps_pool.tile(
                        [P, GT, 512], F32, tag=qkv_tag, name=f"qkv{g}"
                    )
                    for j in range(GT):
                        for ik in range(ND):
                            nc.tensor.matmul(
                                qkv_ps[:, j, 0:QKV_W],
                                lhsT=xg_sb[:, ik, ts(j, P)],
                                rhs=wqkv_sb[:, ik, :],
                                start=(ik == 0),
                                stop=(ik == ND - 1),
                            )
                    # raw q|k out of psum as bf16, tightly packed
                    nc.vector.tensor_copy(
                        qk_raw.rearrange("p n x -> p (n x)"),
                        qkv_ps[:, :, 0 : (NH + 1) * HD],
                    )
                    # copy V out right away (last psum reader)
                    nc.vector.tensor_copy(
                        v_sb[:, ts(g, GT), 0:64], qkv_ps[:, :, 320:384]
                    )

                # rms statistics from the bf16 q|k (norm-preserving rope
                # runs after, so pre-rope stats are exact)
                sq = qk_pool.tile([P, nqk, HD], BF16, tag="sq")
                qk_flat = qk_raw.rearrange("p n x -> p (n x)")
                nc.vector.tensor_mul(
                    sq.rearrange("p n x -> p (n x)"), qk_flat, qk_flat
                )
                ssq = stat_pool.tile([P, nqk], F32, tag="ssq")
                nc.vector.reduce_sum(ssq, sq, axis=mybir.AxisListType.X)
                nc.scalar.activation(
                    out=ssq,
                    in_=ssq,
                    func=mybir.ActivationFunctionType.Sqrt,
                    bias=eps_ap,
                    scale=1.0 / HD,
                )
                rstd = stat_pool.tile([P, nqk], F32, tag="rstd")
                nc.vector.reciprocal(rstd, ssq)
                rstd4 = rstd.rearrange("p (g h) -> p g h", g=GT)
                # fold q_gain into the q rstds
                nc.vector.tensor_mul(rstd4[:, :, 0:NH], rstd4[:, :, 0:NH], qg4)
                # stash k rstd * SCALE for the exp
                nc.scalar.mul(
                    out=rstdk_sb[:, ts(g, GT)],
                    in_=rstd4[:, :, NH : NH + 1].rearrange("p g o -> p (g o)"),
                    mul=SCALE,
                )

                # rope in place on bf16 q|k (rotation commutes with rms
                # scale; bf16 operands hit the 4x DVE modes)
                q1 = qk_raw[:, :, 0 : HD // 2]
                q2 = qk_raw[:, :, HD // 2 : HD]
                cg = cos_sb[:, ts(g, GT), :, :].rearrange("p g h x -> p (g h) x")
                sg = sin_sb[:, ts(g, GT), :, :].rearrange("p g h x -> p (g h) x")
                t_a = qk_pool.tile([P, nqk, HD // 2], BF16, tag="ta")
                t_b = qk_pool.tile([P, nqk, HD // 2], BF16, tag="tb")
                t_c = qk_pool.tile([P, nqk, HD // 2], BF16, tag="tc")
                t_d = qk_pool.tile([P, nqk, HD // 2], BF16, tag="td")
                nc.vector.tensor_mul(t_a, q1, cg)
                nc.vector.tensor_mul(t_b, q2, sg)
                nc.vector.tensor_mul(t_c, q1, sg)
                nc.vector.tensor_mul(t_d, q2, cg)
                nc.vector.tensor_add(q1, t_a, t_b)
                nc.vector.tensor_sub(q2, t_d, t_c)

                # scale q heads by rstd in place (k stays unscaled; its rms
                # scale is folded into the exp)
                for j in range(GT):
                    for h in range(NH):
                        i = j * (NH + 1) + h
                        nc.vector.tensor_scalar_mul(
                            out=qk_raw[:, i, :],
                            in0=qk_raw[:, i, :],
                            scalar1=rstd[:, i : i + 1],
                        )

                # bf16 transposes: q head-pairs and k.  Drains ride the ACT
                # engine for front groups (idle there) and DVE when woven
                # into the attention stream (ACT is the bottleneck there).
                drain = nc.vector.tensor_copy if woven else nc.scalar.copy
                for j in range(GT):
                    it = g * GT + j
                    i0 = j * (NH + 1)
                    tr_tag = "s" if tr_ps_pool is qkv_ps_pool else "trq"
                    for pr in range(2):
                        trq = tr_ps_pool.tile([P, P], BF16, tag=tr_tag)
                        nc.tensor.transpose(
                            trq, qk_raw[:, i0 + 2 * pr : i0 + 2 * pr + 2, :], ident
                        )
                        drain(qT_sb[:, pr, ts(it, P)], trq)
                    trk = tr_ps_pool.tile([P, P], BF16, tag=tr_tag)
                    nc.tensor.transpose(trk[0:64, :], qk_raw[:, i0 + NH, :], ident)
                    drain(kT_sb[0:64, ts(it, P)], trk[0:64, :])
                    # replicate kT into the upper partition half for the
                    # odd-head row-group score matmuls
                    nc.sync.dma_start(
                        out=kT_sb[64:128, ts(it, P)], in_=kT_sb[0:64, ts(it, P)]
                    )

            def emit_chunk(c, s_ps_pool, y_ps_pool):
                """Attention for tq chunk c (needs groups 0..c done)."""
                ntk = (c + 1) * (CW // P)
                y_ps = [
                    y_ps_pool.tile([P, CW], F32, tag="y", name=f"y_c{c}_{h}")
                    for h in range(NH)
                ]
                for tk in range(ntk):
                    dj = tk - 4 * c  # >= 0 on the diagonal block
                    lo = P * dj if dj >= 0 else 0
                    u = u_pool.tile([P, NH, CW], BF16, tag="u")
                    for pr in range(2):
                        s_ps = s_ps_pool.tile([P, 2, CW], F32, tag="s")
                        for hh in range(2):
                            nc.tensor.matmul(
                                s_ps[:, hh, lo:],
                                lhsT=kT_sb[64 * hh : 64 * (hh + 1), ts(tk, P)],
                                rhs=qT_sb[
                                    64 * hh : 64 * (hh + 1),
                                    pr,
                                    c * CW + lo : (c + 1) * CW,
                                ],
                                start=True,
                                stop=True,
                            )
                        nc.scalar.activation(
                            out=u[:, 2 * pr : 2 * pr + 2, lo:],
                            in_=s_ps[:, :, lo:],
                            func=mybir.ActivationFunctionType.Exp,
                            scale=rstdk_sb[:, tk : tk + 1],
                        )
                        if dj >= 0:
                            # multiplicative causal mask on the diagonal
                            # 128-block of u (DVE, off the PE->ACT chain)
                            nc.vector.tensor_mul(
                                u[:, 2 * pr : 2 * pr + 2, lo : lo + P],
                                u[:, 2 * pr : 2 * pr + 2, lo : lo + P],
                                tri2b,
                            )
                    if debug and c == 0 and tk == 0:
                        nc.sync.dma_start(out=dbg_u[:], in_=u)
                    for h in range(NH):
                        # [V | 1] operand: psum rows 0:64 accumulate y^T,
                        # rows 64:128 the broadcast denominator
                        nc.tensor.matmul(
                            y_ps[h][:, lo:],
                            lhsT=v_sb[:, tk, :],
                            rhs=u[:, h, lo:],
                            start=(tk == 0),
                            stop=(tk == ntk - 1),
                            skip_group_check=True,
                        )

                for pr in range(2):
                    he, ho = 2 * pr, 2 * pr + 1
                    dr = r_pool.tile([P, CW], F32, tag="dr")
                    nc.vector.reciprocal(dr[0:64, :], y_ps[he][64:128, :])
                    nc.vector.reciprocal(dr[64:128, :], y_ps[ho][64:128, :])
                    if debug:
                        nc.sync.dma_start(out=dbg_den[c, he], in_=y_ps[he])
                        nc.sync.dma_start(out=dbg_den[c, ho], in_=y_ps[ho])
                    nc.vector.tensor_mul(
                        yT_sb[0:64, pr, ts(c, CW)], y_ps[he][0:64, :], dr[0:64, :]
                    )
                    nc.vector.tensor_mul(
                        yT_sb[64:128, pr, ts(c, CW)],
                        y_ps[ho][0:64, :],
                        dr[64:128, :],
                    )

            def emit_proj(c, proj_ps_pool):
                """Output projection for t-tiles 4c..4c+3, riding the y psum
                slots (tag-shared) so it overlaps the next chunk's
                score/exp stream without touching the s slots."""
                for j in range(GT):
                    it = c * GT + j
                    ob = ob_pool.tile([P, D], BF16, tag="ob")
                    for nh_ in range(2):
                        pj = proj_ps_pool.tile(
                            [P, CW], F32, tag="y", name=f"pj{c}_{j}_{nh_}"
                        )
                        for kt in range(2):
                            nc.tensor.matmul(
                                pj,
                                lhsT=yT_sb[:, kt, ts(it, P)],
                                rhs=wproj_sb[:, kt, ts(nh_, CW)],
                                start=(kt == 0),
                                stop=(kt == 1),
                            )
                        nc.vector.tensor_copy(ob[:, ts(nh_, CW)], pj)
                    nc.sync.dma_start(out=out_d[ts(it, P), :], in_=ob)

            nfront = int(os.environ.get("KERNEL_NFRONT", "4"))
            with (
                tc.tile_pool(name="qkv_ps", bufs=_b("QKV", 1), space="PSUM") as qkv_ps_pool,
                tc.tile_pool(name="tr_ps", bufs=_b("TR", 4), space="PSUM") as tr_ps_pool,
            ):
                for g in range(nfront):
                    emit_group(g, qkv_ps_pool, tr_ps_pool)
            with (
                tc.tile_pool(name="s_ps", bufs=2, space="PSUM") as s_ps_pool,
                tc.tile_pool(name="y_ps", bufs=4, space="PSUM") as y_ps_pool,
            ):
                for c in range(NCHUNK):
                    emit_chunk(c, s_ps_pool, y_ps_pool)
                    if c + nfront < NG:
                        emit_group(
                            c + nfront, s_ps_pool, s_ps_pool, qkv_tag="s"
                        )
                    emit_proj(c, y_ps_pool)

            if debug:
                nc.sync.dma_start(out=dbg_qT[:], in_=qT_sb)
                nc.sync.dma_start(out=dbg_kT[:], in_=kT_sb)
                nc.sync.dma_start(out=dbg_v[:], in_=v_sb)
                nc.sync.dma_start(out=dbg_rk[:], in_=rstdk_sb)
                nc.sync.dma_start(out=dbg_yT[:], in_=yT_sb)

    nc.finalize()
    return nc


_NC_CACHE = {}


def _get_nc(debug=False):
    key = "dbg" if debug else "nc"
    if key not in _NC_CACHE:
        _NC_CACHE[key] = _build_bass(debug=debug)
    return _NC_CACHE[key]


def _make_consts(q_gain_local):
    consts = np.zeros((P, NCONST), dtype=np.float32)
    consts[:, C_QG : C_QG + 16] = np.tile(
        np.asarray(q_gain_local, np.float32)[None, :], (P, GT)
    )
    consts[:, C_EPS] = EPS
    return consts


def _make_blk():
    blk = np.zeros((P, BLK_W), dtype=ml_dtypes.bfloat16)
    blk[:, 0:128] = np.eye(P, dtype=np.float32).astype(ml_dtypes.bfloat16)
    tri = (np.arange(P)[None, :] >= np.arange(P)[:, None]).astype(np.float32)
    blk[:, 128:256] = tri.astype(ml_dtypes.bfloat16)
    blk[:, 256:384] = tri.astype(ml_dtypes.bfloat16)
    return blk


def _rope_tables():
    inv = 1.0 / (
        ROPE_BASE ** (np.arange(0, HD, 2, dtype=np.float32) / HD)
    )
    f = np.arange(T, dtype=np.float32)[:, None] * inv[None, :]
    cos = np.cos(f).astype(ml_dtypes.bfloat16)
    sin = np.sin(f).astype(ml_dtypes.bfloat16)
    # replicate across the 4 q heads + 1 k head (walrus rejects zero-step
    # broadcast APs in TensorTensor, so the broadcast happens host-side)
    cos5 = np.ascontiguousarray(
        np.broadcast_to(cos[:, None, :], (T, NH + 1, HD // 2))
    )
    sin5 = np.ascontiguousarray(
        np.broadcast_to(sin[:, None, :], (T, NH + 1, HD // 2))
    )
    return cos5, sin5


def _make_in_maps(x, Wq, Wk, Wv, Wproj, q_gain):
    x = np.ascontiguousarray(np.asarray(x, np.float32))
    Wq = np.asarray(Wq, np.float32)
    Wk = np.asarray(Wk, np.float32)
    Wv = np.asarray(Wv, np.float32)
    Wproj = np.asarray(Wproj, np.float32)
    q_gain = np.asarray(q_gain, np.float32)
    cos, sin = _rope_tables()
    bf16 = ml_dtypes.bfloat16
    xTs = [np.ascontiguousarray(x[b].T.astype(bf16)) for b in range(B)]
    blk = _make_blk()
    kvw = HKV * HD  # 256 per-core q slice width
    in_maps = []
    for core in range(8):
        b, g = divmod(core, HKV)
        wq = Wq[g * kvw : (g + 1) * kvw]
        wk = Wk[g * HD : (g + 1) * HD]
        wv = Wv[g * HD : (g + 1) * HD]
        wqkvT = np.ascontiguousarray(np.concatenate([wq, wk, wv], 0).T.astype(bf16))
        wprojT = np.ascontiguousarray(
            Wproj[:, g * kvw : (g + 1) * kvw].T.astype(bf16)
        )
        consts = _make_consts(q_gain[g * NH : (g + 1) * NH])
        in_maps.append(
            {
                "xT": xTs[b],
                "wqkvT": wqkvT,
                "wprojT": wprojT,
                "cosT": cos,
                "sinT": sin,
                "consts": consts,
                "blk": blk,
            }
        )
    return in_maps


def run_sharded(inputs, trace=False, debug=False, **kwargs):
    """Run the SPMD kernel; returns (full_output, BassKernelResults)."""
    in_maps = _make_in_maps(**inputs)
    res = run_bass_kernel_spmd(
        _get_nc(debug=debug), in_maps, core_ids=list(range(8)), trace=trace,
        **kwargs
    )
    out = np.zeros((B, T, D), np.float32)
    for core in range(8):
        out[core // HKV] += res.results[core]["outp"].astype(np.float32)
    return out, res


def kernel(x, Wq, Wk, Wv, Wproj, q_gain):
    out, _ = run_sharded(
        dict(x=x, Wq=Wq, Wk=Wk, Wv=Wv, Wproj=Wproj, q_gain=q_gain)
    )
    return out
